# revision 85
# baseline (speedup 1.0000x reference)
"""Causal MHA with RoPE on 8 NeuronCores (Trainium2, Bass/Tile).

Problem: B=1, S=2048, D=1024, H=16 heads, head_dim=64.
Sharding: 2 heads per core (tensor parallel on heads). Out-projection is
row-parallel: each core computes a partial y.T = Wo_local.T @ o_local.T and
the host sums the 8 partials (+ bo).

All PE operands are fp16 (1 cycle/row on TRN2 vs ~3 for fp32r); PSUM
accumulation stays fp32.  The head_dim axis is permuted so that RoPE
rotate-half pairs (d, d+32) sit on adjacent partitions (2d', 2d'+1): the
half-swap is then a DVE stream_shuffle (mask i^1) instead of 4 SBUF-SBUF
partition-block DMAs per chunk.  The permutation is absorbed into the rows
of Wq/Wk and the cos/sin tables on the host; q.k dot products are invariant.

Per-core device layout (partitions 0-63 = head A dims, 64-127 = head B):
  xT_sb   [128, 8, 2048] f16 : x.T (host-pretransposed), d on partitions
  qk_rope [128, 2, 2048] f16 : RoPE'd q.T (g=0) and k.T (g=1)
  v_nat   [128, 16, 130] f16 : v in natural [s, e] layout + ones cols
                               (cols 0-63 vA, 64 ones, 65-128 vB, 129 ones)
  scores.T tiles [j=128, i<=512] psum f32 per (i-chunk, j-tile), causal only
  o_aug   [65, 512] psum f32 per (head, i-chunk): rows 0-63 = (e@v).T,
          row 64 = softmax denominator (from the ones column)
  y       staged [128, 8, 512] f16 per chunk, one DMA trigger per chunk

Engine placement notes (hard-won): the causal mask is a 0/1 f16 multiply on
DVE post-exp (keeping gpsimd broadcast-only — mixing affine_select with
partition_broadcast on gpsimd forces a ucode lib swap per chunk boundary,
~3-6us each); PSUM->SBUF casts ride the otherwise-idle ScalarE in phase 1
and split across DVE+ACT in the drain; softmax exp is the only hard ScalarE
load (~40us).  Cross-engine deps lower to scheduled-order counters, so
stalls are fixed by removing latency from producer chains, not reordering.
"""

import functools
import os

os.environ.setdefault("JAX_PLATFORMS", "")

import numpy as np

import concourse.bacc as bacc
import concourse.mybir as mybir
import concourse.tile as tile
from concourse.bass import ds, ts
from concourse.bass_utils import run_bass_kernel_spmd

F32 = mybir.dt.float32
F16 = mybir.dt.float16
AF = mybir.ActivationFunctionType
OP = mybir.AluOpType

B, S, D, H = 1, 2048, 1024, 16
HD = D // H          # 64
NC = 8               # cores
HPC = H // NC        # 2 heads per core
P = 128              # partitions
CHUNK = 512          # i-chunk (moving free dim)
NCHUNK = S // CHUNK  # 4
NJT = S // P         # 16 j-tiles
KO = D // P          # 8 k-subtiles for projections

# head-dim permutation: partition p (within a 64-dim head block) holds
# hd index pi(p) = (p % 2)*32 + p//2, so RoPE pairs (d, d+32) are the
# adjacent partitions (2d', 2d'+1).
_PI64 = np.array([(p % 2) * 32 + p // 2 for p in range(HD)], np.int64)
_PIFULL = np.concatenate([_PI64, HD + _PI64])  # for a 128-row (2-head) block
# stream_shuffle mask: swap adjacent partitions within each 32-quadrant
_SWAP_MASK = [i ^ 1 for i in range(32)]
# v-dim permutation: vT partition r holds v-dim (r%2)*64 + r//2 so that the
# XBAR dma transpose (out[p, g, c] = in[2c+g, p]) lands head-major in v_nat
_PIV = np.array([(r % 2) * HD + r // 2 for r in range(P)], np.int64)


def _rope_tables():
    inv_freq = 1.0 / (10000.0 ** (np.arange(0, HD, 2, dtype=np.float32) / HD))  # [32]
    t = np.arange(S, dtype=np.float32)
    freqs = np.einsum("s,f->sf", t, inv_freq)  # [S, 32]
    cos_h = np.cos(freqs).astype(np.float32)   # [S, 32] (same for both halves)
    sin_h = np.sin(freqs).astype(np.float32)
    # partition p holds hd = pi(p%64); table index = p//2 % 32;
    # swapped-term sign: -1 for first half (hd<32, i.e. p even), +1 else
    cosP = np.empty((P, S), np.float32)
    sinP = np.empty((P, S), np.float32)
    for p in range(P):
        q = p % HD
        cosP[p] = cos_h[:, q // 2]
        sinP[p] = -sin_h[:, q // 2] if (q % 2 == 0) else sin_h[:, q // 2]
    return cosP, sinP


def _persist_pool(tc):
    return tc.tile_pool(name="persist", bufs=1)


def _build(with_qk_bias: bool, with_v_bias: bool = True, debug: bool = False):
    nc = bacc.Bacc("TRN2", target_bir_lowering=False, debug=False, num_devices=NC)

    xT_d = nc.dram_tensor("xT", [D, S], F16, kind="ExternalInput").ap()
    wq_d = nc.dram_tensor("wqT", [P, KO, P], F16, kind="ExternalInput").ap()
    wk_d = nc.dram_tensor("wkT", [P, KO, P], F16, kind="ExternalInput").ap()
    wv_d = nc.dram_tensor("wvT", [P, KO, P], F16, kind="ExternalInput").ap()
    wo_d = nc.dram_tensor("woT", [P, D], F16, kind="ExternalInput").ap()
    bv_d = nc.dram_tensor("bv", [P, 1], F32, kind="ExternalInput").ap()
    cos_d = nc.dram_tensor("cosP", [P, S], F16, kind="ExternalInput").ap()
    sin_d = nc.dram_tensor("sinP", [P, S], F16, kind="ExternalInput").ap()
    idn_d = nc.dram_tensor("ident", [P, P], F16, kind="ExternalInput").ap()
    msk_d = nc.dram_tensor("tmask", [P, P], F16, kind="ExternalInput").ap()
    if with_qk_bias:
        rb_d = nc.dram_tensor("ropeB", [P, 2, S], F16, kind="ExternalInput").ap()
    yT_d = nc.dram_tensor("yT", [D, S], F16, kind="ExternalOutput").ap()
    if debug:
        dbg_qk = nc.dram_tensor("dbg_qk", [P, 2, S], F16, kind="ExternalOutput").ap()
        dbg_v = nc.dram_tensor("dbg_v", [P, NJT, 2 * HD + 2], F16, kind="ExternalOutput").ap()
        dbg_o = nc.dram_tensor("dbg_o", [P, S], F16, kind="ExternalOutput").ap()

    with tile.TileContext(nc) as tc, _persist_pool(tc) as pp:
        # ---- persistent SBUF tiles ----
        xT_sb = pp.tile([P, KO, S], F16, name="xT_sb", tag="xT_sb")
        wq_sb = pp.tile([P, KO, P], F16, name="wq_sb")
        wk_sb = pp.tile([P, KO, P], F16, name="wk_sb")
        wv_sb = pp.tile([P, KO, P], F16, name="wv_sb")
        wo_sb = pp.tile([P, D], F16, name="wo_sb")
        bv_sb = pp.tile([P, 8], F32, name="bv_sb")
        cos_sb = pp.tile([P, S], F16, name="cos_sb")
        sin_sb = pp.tile([P, S], F16, name="sin_sb")
        idn_sb = pp.tile([P, P], F16, name="idn_sb")
        qk_rope = pp.tile([P, 2, S], F16, name="qk_rope")
        vT_sb = pp.tile([P, S], F16, name="vT_sb")
        v_nat = pp.tile([P, NJT, 2 * HD + 2], F16, name="v_nat")
        o_nT = pp.tile([P, S], F16, name="o_nT")
        msk_sb = pp.tile([P, P], F16, name="msk_sb")
        if with_qk_bias:
            rb_sb = pp.tile([P, 2, S], F16, name="rb_sb")

        xTr = xT_d.rearrange("(o p) s -> p o s", p=P)
        yTr = yT_d.rearrange("(t p) s -> p t s", p=P)
        # startup loads: the first-needed tensors (wv+x0 pair for the v(0)
        # projection, then wq) lead so the PE can start earliest
        nc.sync.dma_start(wv_sb[:, 0:2, :], wv_d[:, 0:2, :])
        nc.sync.dma_start(
            xT_sb[:, 0:2, ts(0, CHUNK)], xTr[:, 0:2, ts(0, CHUNK)])
        nc.sync.dma_start(wv_sb[:, 2:KO, :], wv_d[:, 2:KO, :])
        nc.sync.dma_start(
            xT_sb[:, 2:4, ts(0, CHUNK)], xTr[:, 2:4, ts(0, CHUNK)])
        nc.sync.dma_start(wq_sb[:], wq_d)
        nc.sync.dma_start(
            xT_sb[:, 4:6, ts(0, CHUNK)], xTr[:, 4:6, ts(0, CHUNK)])
        nc.sync.dma_start(
            xT_sb[:, 6:KO, ts(0, CHUNK)], xTr[:, 6:KO, ts(0, CHUNK)])
        nc.sync.dma_start(wk_sb[:], wk_d)
        nc.sync.dma_start(idn_sb[:], idn_d)
        nc.sync.dma_start(msk_sb[:], msk_d)
        nc.sync.dma_start(cos_sb[:], cos_d)
        nc.sync.dma_start(sin_sb[:], sin_d)
        for ci in range(1, NCHUNK):
            nc.sync.dma_start(
                xT_sb[:, 0:4, ts(ci, CHUNK)], xTr[:, 0:4, ts(ci, CHUNK)])
            nc.sync.dma_start(
                xT_sb[:, 4:KO, ts(ci, CHUNK)], xTr[:, 4:KO, ts(ci, CHUNK)])
        nc.sync.dma_start(wo_sb[:], wo_d)
        if with_v_bias:
            nc.sync.dma_start(bv_sb[:, 0:1], bv_d)
        if with_qk_bias:
            nc.sync.dma_start(rb_sb[:], rb_d)

        scale = (1.0 / np.sqrt(HD)).item()

        with tc.tile_pool(name="sb_all", bufs=3) as sb_all:
            sb_b = sb_e = sb_r = sb_y = sb_all

            def b_qk(ci, pool):
                s0 = ci * CHUNK
                ps_qk = pool.tile([P, 2 * CHUNK], F32, tag="qk", name=f"qk_{ci}")
                for o in range(KO):
                    nc.tensor.matmul(
                        ps_qk[:, 0:CHUNK], wq_sb[:, o, :], xT_sb[:, o, ds(s0, CHUNK)],
                        start=(o == 0), stop=(o == KO - 1),
                    )
                for o in range(KO):
                    nc.tensor.matmul(
                        ps_qk[:, CHUNK:2 * CHUNK], wk_sb[:, o, :], xT_sb[:, o, ds(s0, CHUNK)],
                        start=(o == 0), stop=(o == KO - 1),
                    )
                qraw = sb_b.tile([P, 2 * CHUNK], F16, tag="qraw", name=f"qraw_{ci}")
                # late chunks cast on DVE so the ScalarE queue reaches the
                # first exps as soon as their (bank-hoisted) scores are ready
                if ci < 2:
                    nc.scalar.copy(out=qraw[:], in_=ps_qk[:])
                else:
                    nc.vector.tensor_copy(out=qraw[:], in_=ps_qk[:])
                qsw = sb_b.tile([P, 2 * CHUNK], F16, tag="qsw", name=f"qsw_{ci}")
                nc.vector.stream_shuffle(out=qsw[:], in_=qraw[:], mask=_SWAP_MASK)
                qsw3 = qsw.rearrange("p (g n) -> p g n", g=2)
                qraw3 = qraw.rearrange("p (g n) -> p g n", g=2)
                m1 = sb_b.tile([P, 2 * CHUNK], F16, tag="m1", name=f"m1_{ci}")
                m13 = m1.rearrange("p (g n) -> p g n", g=2)
                cosd = cos_sb[:, None, ds(s0, CHUNK)].to_broadcast([P, 2, CHUNK])
                sind = sin_sb[:, None, ds(s0, CHUNK)].to_broadcast([P, 2, CHUNK])
                nc.vector.tensor_tensor(out=m13[:], in0=qraw3[:], in1=cosd, op=OP.mult)
                nc.vector.tensor_tensor(out=qsw3[:], in0=qsw3[:], in1=sind, op=OP.mult)
                rout = qk_rope[:, :, ds(s0, CHUNK)]
                if with_qk_bias:
                    tmp = sb_b.tile([P, 2 * CHUNK], F16, tag="tmp", name=f"tmp_{ci}")
                    tmp3 = tmp.rearrange("p (g n) -> p g n", g=2)
                    nc.vector.tensor_tensor(out=tmp3[:], in0=m13[:], in1=qsw3[:], op=OP.add)
                    nc.vector.tensor_tensor(
                        out=rout, in0=tmp3[:], in1=rb_sb[:, :, ds(s0, CHUNK)], op=OP.add)
                else:
                    nc.vector.tensor_tensor(out=rout, in0=m13[:], in1=qsw3[:], op=OP.add)

            def b_v(ci, pool):
                s0 = ci * CHUNK
                ps_v = pool.tile([P, CHUNK], F32, tag="v", name=f"v_{ci}")
                for o in range(KO):
                    nc.tensor.matmul(
                        ps_v[:], wv_sb[:, o, :], xT_sb[:, o, ds(s0, CHUNK)],
                        start=(o == 0), stop=(o == KO - 1),
                    )
                if with_v_bias:
                    nc.vector.tensor_scalar(
                        out=vT_sb[:, ds(s0, CHUNK)], in0=ps_v[:],
                        scalar1=bv_sb[:, 0:1], scalar2=None, op0=OP.add,
                    )
                else:
                    nc.scalar.copy(out=vT_sb[:, ds(s0, CHUNK)], in_=ps_v[:])
                if ci == 0:
                    nc.vector.memset(v_nat[:, :, HD:HD + 1], 1.0)
                    nc.vector.memset(v_nat[:, :, 2 * HD + 1:2 * HD + 2], 1.0)
            def b_tr(ci, pool):
                # v_nat copies ride the idle ScalarE in phase 1, except the
                # final tiles which stay on DVE so the first exps are not
                # queued behind them on ScalarE
                for tj in range(4 * ci, 4 * ci + 4):
                    ps_t = pool.tile([P, P], F16, tag="tr", name=f"tr_{tj}")
                    nc.tensor.transpose(ps_t[:], vT_sb[:, ts(tj, P)], idn_sb[:])
                    nc.scalar.copy(out=v_nat[:, tj, 0:HD], in_=ps_t[:, 0:HD])
                    nc.scalar.copy(
                        out=v_nat[:, tj, HD + 1:2 * HD + 1], in_=ps_t[:, HD:2 * HD])

            def scores(ci, tj, _unused=None):
                """scores.T matmuls + exp (+ causal mask) for one (ci, tj)."""
                i0 = ci * CHUNK
                jlo = tj * P
                c0 = max(i0, jlo)
                n = i0 + CHUNK - c0
                off = c0 - i0
                ps_s = ps_sp.tile([P, 2 * CHUNK], F32, tag="s", name=f"s_{ci}_{tj}")
                for h in range(HPC):
                    nc.tensor.matmul(
                        ps_s[:, ds(h * CHUNK + off, n)],
                        qk_rope[ds(64 * h, 64), 1, ds(jlo, P)],
                        qk_rope[ds(64 * h, 64), 0, ds(c0, n)],
                        start=True, stop=True, tile_position=(64 * h, 0),
                    )
                s3 = ps_s.rearrange("p (g n) -> p g n", g=2)
                e_t = sb_e.tile([P, 2 * CHUNK], F16, tag="e", name=f"e_{ci}_{tj}")
                e3 = e_t.rearrange("p (g n) -> p g n", g=2)
                nc.scalar.activation(
                    e3[:, :, ds(off, n)], s3[:, :, ds(off, n)], AF.Exp, scale=scale)
                if jlo >= i0:
                    # causal mask: multiply the diagonal square by a 0/1
                    # triangle on DVE (gpsimd stays broadcast-only, avoiding
                    # its ucode lib thrash)
                    sq = e3[:, :, ds(off, P)]
                    mskb = msk_sb[:, None, :].to_broadcast([P, 2, P])
                    nc.vector.tensor_tensor(out=sq, in0=sq, in1=mskb, op=OP.mult)
                return e_t, off, n

            def attnv(ci, tj, e_t, off, n, o_ps):
                ntj = 4 * ci + 4
                for h in range(HPC):
                    nc.tensor.matmul(
                        o_ps[h][:, ds(off, n)],
                        v_nat[:, tj, ds(h * (HD + 1), HD + 1)],
                        e_t[:, ds(h * CHUNK + off, n)],
                        start=(tj == 0), stop=(tj == ntj - 1),
                    )

            def norm(ci, o_ps):
                # denominator chain gates the out-proj: interleave the two
                # heads' ops so DVE/gpsimd stages pipeline; for the last
                # chunk the idle ScalarE does the PSUM row copies
                i0 = ci * CHUNK
                last = ci == NCHUNK - 1
                with tc.high_priority():
                    dcps, r1s, r64s = [], [], []
                    for h in range(HPC):
                        dcp = sb_r.tile([1, CHUNK], F32, tag="dcp", name=f"dcp_{ci}_{h}")
                        if last:
                            nc.scalar.copy(out=dcp[:], in_=o_ps[h][HD:HD + 1, :])
                        else:
                            nc.vector.tensor_copy(out=dcp[:], in_=o_ps[h][HD:HD + 1, :])
                        dcps.append(dcp)
                    for h in range(HPC):
                        r1 = sb_r.tile([1, CHUNK], F32, tag="r1", name=f"r1_{ci}_{h}")
                        nc.vector.reciprocal_approx_fast(out=r1[:], in_=dcps[h][:])
                        r1s.append(r1)
                    for h in range(HPC):
                        r64 = sb_r.tile([HD, CHUNK], F32, tag="r64", name=f"r64_{ci}_{h}")
                        nc.gpsimd.partition_broadcast(r64[:], r1s[h][:])
                        r64s.append(r64)
                    for h in range(HPC):
                        nc.vector.tensor_tensor(
                            out=o_nT[ds(64 * h, 64), ds(i0, CHUNK)], in0=o_ps[h][0:HD, :],
                            in1=r64s[h][:], op=OP.mult)

            ystages = {}
            ymode = {}  # per-chunk: True = split casts + per-2dt DMA

            def outproj_unit(ci, dt, drain=False):
                i0 = ci * CHUNK
                last = ymode.setdefault(ci, drain or ci == NCHUNK - 1)
                ps_y = ps_yp.tile([P, CHUNK], F32, tag="y", name=f"y_{ci}_{dt}")
                nc.tensor.matmul(
                    ps_y[:], wo_sb[:, ts(dt, P)],
                    o_nT[:, ds(i0, CHUNK)], start=True, stop=True)
                if ci not in ystages:
                    ystages[ci] = sb_y.tile(
                        [P, KO, CHUNK], F16, tag="ystage", name=f"ystage_{ci}")
                yst = ystages[ci]
                # drain/last chunk: split each cast column-wise across
                # DVE+ACT and DMA in 2-dt slices so the tail isn't
                # serialized on one engine+queue
                if last:
                    nc.vector.tensor_copy(
                        out=yst[:, dt, 0:CHUNK // 2], in_=ps_y[:, 0:CHUNK // 2])
                    nc.scalar.copy(
                        out=yst[:, dt, CHUNK // 2:], in_=ps_y[:, CHUNK // 2:])
                else:
                    nc.vector.tensor_copy(out=yst[:, dt, :], in_=ps_y[:])
                if last:
                    if dt % 2 == 1:
                        nc.sync.dma_start(
                            yTr[:, ds(dt - 1, 2), ds(i0, CHUNK)],
                            yst[:, ds(dt - 1, 2), :])
                elif dt == KO - 1:
                    nc.sync.dma_start(yTr[:, :, ds(i0, CHUNK)], yst[:])

            # Flat software-pipelined attention across all (ci, tj) pairs:
            # scores(k+1) is emitted before attnv(k) so the PE never stalls
            # on the ScalarE exp; out-proj of chunk ci is delayed past the
            # first two scores of chunk ci+1 for the same reason.
            # v runs a full chunk ahead of qk, and tr(ci) is emitted ~24
            # matmuls after v(ci): the DVE v-copy feeding each transpose is
            # long done by the time the PE queue reaches it, in both the
            # scheduler's cost model and on hardware
            # all transposes complete before qk(2)/qk(3): the phase-1 tail
            # then has no PSUM readers left except qraw(3), so the first
            # scores tile's bank-reuse fence resolves right as phase 1 ends
            with tc.tile_pool(name="ps_b", bufs=2, space="PSUM") as ps_bb:
                b_v(0, ps_bb)
                b_v(1, ps_bb)
                b_qk(0, ps_bb)
                b_tr(0, ps_bb)
                b_v(2, ps_bb)
                b_qk(1, ps_bb)
                b_tr(1, ps_bb)
                b_v(3, ps_bb)
                b_tr(2, ps_bb)
                b_tr(3, ps_bb)
                b_qk(2, ps_bb)
                b_qk(3, ps_bb)
            cpools = (tc.tile_pool(name="ps_att", bufs=2, space="PSUM"),)
            ps_sp = ps_op = ps_yp = cpools[0].__enter__()

            pairs = [(ci, tj) for ci in range(NCHUNK) for tj in range(4 * ci + 4)]
            proj_q = []
            o_tiles = {}
            pend_av = []         # [(ci, tj, e_t, off, n)] awaiting attnv (lag 2)
            pend_proj = []       # ci's whose outproj is due
            emitted_since_proj = 0

            def do_attnv(ent):
                pci, ptj, pe_t, poff, pn = ent
                if pci not in o_tiles:
                    o_tiles[pci] = [
                        ps_op.tile([HD + 1, CHUNK], F32, tag="o", name=f"o_{pci}_{h}")
                        for h in range(HPC)]
                attnv(pci, ptj, pe_t, poff, pn, o_tiles[pci])
                if ptj == 4 * pci + 3:
                    norm(pci, o_tiles[pci])
                    pend_proj.append(pci)
                    return True
                return False

            for k, (ci, tj) in enumerate(pairs):
                cur = scores(ci, tj, None)
                emitted_since_proj += 1
                remaining = len(pairs) - k
                # release a pending chunk's out-proj once the norm chain has
                # had ~10 pairs of headroom — or immediately when the pair
                # stream is about to end, so the final chunk's norm never
                # queues behind these casts on the DVE
                if pend_proj and (emitted_since_proj >= 10 or remaining <= 10):
                    pc = pend_proj.pop(0)
                    proj_q.extend((pc, dt) for dt in range(D // P))
                if proj_q:
                    outproj_unit(*proj_q.pop(0))
                    # catch up so the queue drains well before the chunk ends
                    if len(proj_q) > 4 or (remaining <= 6 and proj_q):
                        outproj_unit(*proj_q.pop(0))
                if len(pend_av) >= 1:
                    if do_attnv(pend_av.pop(0)):
                        emitted_since_proj = 0
                pend_av.append((ci, tj, *cur))
            for ent in pend_av:
                do_attnv(ent)
                while proj_q:
                    pc, pdt = proj_q.pop(0)
                    outproj_unit(pc, pdt, drain=True)
            for c in pend_proj:
                for dt in range(D // P):
                    outproj_unit(c, dt, drain=True)
            for cp in reversed(cpools):
                cp.__exit__(None, None, None)

        if debug:
            nc.sync.dma_start(dbg_qk, qk_rope[:])
            nc.sync.dma_start(dbg_v, v_nat[:])
            nc.sync.dma_start(dbg_o, o_nT[:])

    nc.compile()
    return nc


@functools.lru_cache(maxsize=4)
def _get_nc(with_qk_bias: bool, with_v_bias: bool = True, debug: bool = False):
    return _build(with_qk_bias, with_v_bias, debug)


def kernel(x, Wq, bq, Wk, bk, Wv, bv, Wo, bo):
    x = np.asarray(x, np.float32)
    Wq, bq = np.asarray(Wq, np.float32), np.asarray(bq, np.float32)
    Wk, bk = np.asarray(Wk, np.float32), np.asarray(bk, np.float32)
    Wv, bv = np.asarray(Wv, np.float32), np.asarray(bv, np.float32)
    Wo, bo = np.asarray(Wo, np.float32), np.asarray(bo, np.float32)

    cosP, sinP = _rope_tables()
    cosP16 = cosP.astype(np.float16)
    sinP16 = sinP.astype(np.float16)
    xT = np.ascontiguousarray(x[0].T).astype(np.float16)  # [D, S]
    ident = np.eye(P, dtype=np.float16)
    jj, ii = np.meshgrid(np.arange(P), np.arange(P), indexing="ij")
    tmask = (jj <= ii).astype(np.float16)

    with_qk_bias = bool(np.abs(bq).max() > 0 or np.abs(bk).max() > 0)
    with_v_bias = bool(np.abs(bv).max() > 0)
    nc = _get_nc(with_qk_bias, with_v_bias, bool(getattr(kernel, 'debug', False)))

    def _wprep(W, r0, perm=None):
        Wl = W[r0:r0 + P]
        if perm is not None:
            Wl = Wl[perm]
        return np.ascontiguousarray(
            Wl.T.reshape(KO, P, P).transpose(1, 0, 2)).astype(np.float16)

    in_maps = []
    for c in range(NC):
        r0 = c * P  # this core's 128 rows of Wq/Wk/Wv, cols of Wo
        m = {
            "xT": xT,
            "wqT": _wprep(Wq, r0, _PIFULL),
            "wkT": _wprep(Wk, r0, _PIFULL),
            "wvT": _wprep(Wv, r0),
            "woT": np.ascontiguousarray(Wo[:, r0:r0 + P].T).astype(np.float16),
            "bv": np.ascontiguousarray(bv[r0:r0 + P, None]).astype(np.float32),
            "cosP": cosP16,
            "sinP": sinP16,
            "ident": ident,
            "tmask": tmask,
        }
        if with_qk_bias:
            # rope(b)[p, s] = b[p]*cos + b[p^1]*sin' (RoPE is linear)
            rb = np.empty((P, 2, S), np.float32)
            for g, b_ in enumerate((bq, bk)):
                bc = b_[r0:r0 + P][_PIFULL]
                sw = bc.reshape(HD, 2)[:, ::-1].reshape(P)
                rb[:, g, :] = bc[:, None] * cosP + sw[:, None] * sinP
            m["ropeB"] = rb.astype(np.float16)
        in_maps.append(m)

    res = run_bass_kernel_spmd(
        nc, in_maps, core_ids=list(range(NC)),
        trace=bool(getattr(kernel, "trace", False)),
    )
    kernel.last_result = res
    y = np.zeros((S, D), np.float32)
    for c in range(NC):
        y += res.results[c]["yT"].astype(np.float32).T
    y += bo
    return y[None].astype(np.float32)
